# revision 3
# baseline (speedup 1.0000x reference)
"""Causal multi-head attention block (GPT-style) on 8 Trainium2 NeuronCores.

Sharding: 8 cores = 4 batches x 2 head-groups. Core c handles batch c//2 and
heads [6*(c%2), 6*(c%2)+6) of 12. Each core computes qkv projection, causal
attention and its partial output projection; the host sums the two partials
per batch and adds b_proj.

All matmuls run in float32r (TF32-like: ~1.7e-4 max rel err, full PE rate at
N>=256). q/k are produced feature-major ([d, T]) so QK^T needs no transposes;
v is position-major with a fused ones-column so the PV matmul also emits the
softmax denominator. Softmax skips max-subtraction (logits are ~N(0,1); exp
overflow impossible in fp32). Causality: per key-chunk column ranges plus one
triangular-mask multiply on the diagonal 128x128 sub-block.
"""

import numpy as np

B, T, C = 4, 2048, 768
H = 12
DH = 64
HPC = 6          # heads per core
NCORES = 8
QT = 512         # query tile
NQT = T // QT    # 4
NKC = T // 128   # 16 key chunks
FV = HPC * (DH + 1)  # 390: per-head 64 v cols + 1 ones col

_CACHE = {}


def _build():
    import sys
    if '/opt/trn_rl_repo' not in sys.path:
        sys.path.insert(0, '/opt/trn_rl_repo')
    import concourse.tile as tile
    from concourse import bacc, mybir

    F32 = mybir.dt.float32
    F32R = mybir.dt.float32r
    EXP = mybir.ActivationFunctionType.Exp

    nc = bacc.Bacc("TRN2", target_bir_lowering=False, debug=False,
                   num_devices=NCORES)

    xt_ext = nc.dram_tensor("xt", [C, T], F32R, kind="ExternalInput").ap()
    wqk_ext = nc.dram_tensor("wqk", [C, 768], F32R, kind="ExternalInput").ap()
    bqk_ext = nc.dram_tensor("bqk", [768], F32, kind="ExternalInput").ap()
    wv_ext = nc.dram_tensor("wv", [C, FV], F32R, kind="ExternalInput").ap()
    wvb_ext = nc.dram_tensor("wvb", [1, FV], F32R, kind="ExternalInput").ap()
    wp_ext = nc.dram_tensor("wp", [HPC * DH, 768], F32R, kind="ExternalInput").ap()
    tri_ext = nc.dram_tensor("tri", [128, 128], F32, kind="ExternalInput").ap()
    out_ext = nc.dram_tensor("out", [T, 768], F32, kind="ExternalOutput").ap()

    xt_r = xt_ext.rearrange("(c p) n -> p c n", p=128)      # [128, 6, 2048]
    wqk_r = wqk_ext.rearrange("(c p) f -> p c f", p=128)    # [128, 6, 768]
    bqk_r = bqk_ext.rearrange("(c p) -> p c", p=128)        # [128, 6]
    wv_r = wv_ext.rearrange("(c p) f -> p c f", p=128)      # [128, 6, 390]
    wp_r = wp_ext.rearrange("(c p) f -> p c f", p=128)      # [128, 3, 768]

    with tile.TileContext(nc) as tc:
        import contextlib
        stack = contextlib.ExitStack()
        with stack:
            const = stack.enter_context(tc.tile_pool(name="const", bufs=1))
            big = stack.enter_context(tc.tile_pool(name="big", bufs=1))
            xtp = stack.enter_context(tc.tile_pool(name="xtp", bufs=12))
            expp = stack.enter_context(tc.tile_pool(name="expp", bufs=3))
            work = stack.enter_context(tc.tile_pool(name="work", bufs=2))
            psA = stack.enter_context(tc.tile_pool(name="psA", bufs=3, space="PSUM"))
            psY = stack.enter_context(tc.tile_pool(name="psY", bufs=2, space="PSUM"))
            psM = stack.enter_context(tc.tile_pool(name="psM", bufs=3, space="PSUM"))

            # ---- constants / weights ----
            wqk_t = const.tile([128, 6, 768], F32R, tag="wqk")
            nc.sync.dma_start(out=wqk_t, in_=wqk_r)
            bqk_t = const.tile([128, 6], F32, tag="bqk")
            nc.sync.dma_start(out=bqk_t, in_=bqk_r)
            wv_t = const.tile([128, 6, FV], F32R, tag="wv")
            nc.sync.dma_start(out=wv_t, in_=wv_r)
            wvb_t = const.tile([1, FV], F32R, tag="wvb")
            nc.sync.dma_start(out=wvb_t, in_=wvb_ext)
            wp_t = const.tile([128, 3, 768], F32R, tag="wp")
            nc.sync.dma_start(out=wp_t, in_=wp_r)
            tri_t = const.tile([128, 128], F32, tag="tri")
            nc.sync.dma_start(out=tri_t, in_=tri_ext)
            ones_f32 = const.tile([1, T], F32, tag="ones_f32")
            nc.vector.memset(ones_f32, 1.0)
            ones_row = const.tile([1, T], F32R, tag="ones_row")
            nc.vector.tensor_copy(out=ones_row, in_=ones_f32)
            ones64 = const.tile([1, 64], F32R, tag="ones64")
            nc.vector.tensor_copy(out=ones64, in_=ones_f32[:, 0:64])

            # ---- persistent activations ----
            # qkT[fc]: fc 0..2 = q heads (head h -> tile h//2, partitions
            # 64*(h%2)..), fc 3..5 = k heads. All feature-major [128, T].
            qkT = [big.tile([128, T], F32R, tag=f"qkT{fc}", name=f"qkT{fc}") for fc in range(6)]
            v_t = [big.tile([128, FV], F32R, tag=f"v{kc}", name=f"v{kc}") for kc in range(NKC)]
            yT = [big.tile([128, T], F32R, tag=f"yT{kc}", name=f"yT{kc}") for kc in range(3)]

            for qt in range(NQT):
                # ---- phase B: qk^T projection for this query tile ----
                xt_tiles = []
                for cc in range(6):
                    xt_tl = xtp.tile([128, QT], F32R, tag="xt")
                    nc.sync.dma_start(out=xt_tl, in_=xt_r[:, cc, qt * QT:(qt + 1) * QT])
                    xt_tiles.append(xt_tl)
                for fc in range(6):
                    ps = psM.tile([128, QT], F32, tag="mm")
                    for cc in range(6):
                        nc.tensor.matmul(out=ps,
                                         lhsT=wqk_t[:, cc, fc * 128:(fc + 1) * 128],
                                         rhs=xt_tiles[cc],
                                         start=(cc == 0), stop=(cc == 5))
                    nc.vector.tensor_scalar(out=qkT[fc][:, qt * QT:(qt + 1) * QT],
                                            in0=ps, scalar1=bqk_t[:, fc:fc + 1],
                                            scalar2=None, op0=mybir.AluOpType.add)

                # ---- phase C: v projection (position-major, + ones col) ----
                for tv in range(4 * qt, 4 * qt + 4):
                    ps = psM.tile([128, FV], F32, tag="mm")
                    for cc in range(6):
                        nc.tensor.matmul(out=ps,
                                         lhsT=xt_tiles[cc][:, (tv - 4 * qt) * 128:(tv - 4 * qt + 1) * 128],
                                         rhs=wv_t[:, cc, :],
                                         start=(cc == 0), stop=False)
                    nc.tensor.matmul(out=ps,
                                     lhsT=ones_row[:, tv * 128:(tv + 1) * 128],
                                     rhs=wvb_t,
                                     start=False, stop=True)
                    nc.vector.tensor_copy(out=v_t[tv], in_=ps)

                # ---- phase D: attention for all heads, this query tile ----
                for h in range(HPC):
                    po = 64 * (h % 2)
                    q_ap = qkT[h // 2][po:po + 64, qt * QT:(qt + 1) * QT]
                    psum_y = psY.tile([128, QT], F32, tag="y")
                    nkc = 4 * qt + 4
                    for kcg in range(nkc):
                        m = kcg - 4 * qt
                        lo = 128 * m if m >= 0 else 0
                        ps_att = psA.tile([128, QT], F32, tag="att")
                        nc.tensor.matmul(
                            out=ps_att[:, lo:QT],
                            lhsT=qkT[3 + h // 2][po:po + 64, kcg * 128:(kcg + 1) * 128],
                            rhs=q_ap[:, lo:QT],
                            start=True, stop=True)
                        e_t = expp.tile([128, QT], F32R, tag="expT")
                        nc.scalar.activation(out=e_t[:, lo:QT], in_=ps_att[:, lo:QT],
                                             func=EXP)
                        if m >= 0:
                            nc.vector.tensor_mul(e_t[:, lo:lo + 128],
                                                 e_t[:, lo:lo + 128], tri_t)
                        nc.tensor.matmul(
                            out=psum_y[0:65, lo:QT],
                            lhsT=v_t[kcg][:, 65 * h:65 * h + 65],
                            rhs=e_t[:, lo:QT],
                            start=(kcg == 0), stop=(kcg == nkc - 1),
                            skip_group_check=True)
                    # normalize: broadcast den via ones outer product, recip, mul
                    den_sb = work.tile([1, QT], F32R, tag="den")
                    nc.vector.tensor_copy(out=den_sb, in_=psum_y[64:65, :])
                    ps_bc = psA.tile([64, QT], F32, tag="att")
                    nc.tensor.matmul(out=ps_bc, lhsT=ones64, rhs=den_sb,
                                     start=True, stop=True)
                    recb = work.tile([64, QT], F32, tag="recb")
                    nc.vector.reciprocal_approx_fast(out=recb, in_=ps_bc)
                    nc.vector.tensor_mul(
                        yT[h // 2][po:po + 64, qt * QT:(qt + 1) * QT],
                        psum_y[0:64, :], recb)

                # ---- phase E: output projection for this query tile ----
                for tt in range(4 * qt, 4 * qt + 4):
                    osb = work.tile([128, 768], F32, tag="osb")
                    for no in range(2):
                        ne = 512 if no == 0 else 256
                        ps = psM.tile([128, QT], F32, tag="mm")
                        for kc in range(3):
                            nc.tensor.matmul(
                                out=ps[:, 0:ne],
                                lhsT=yT[kc][:, tt * 128:(tt + 1) * 128],
                                rhs=wp_t[:, kc, no * 512:no * 512 + ne],
                                start=(kc == 0), stop=(kc == 2))
                        nc.vector.tensor_copy(out=osb[:, no * 512:no * 512 + ne],
                                              in_=ps[:, 0:ne])
                    nc.sync.dma_start(out=out_ext[tt * 128:(tt + 1) * 128, :],
                                      in_=osb)

    nc.compile()
    return nc


def _get_nc():
    if 'nc' not in _CACHE:
        _CACHE['nc'] = _build()
    return _CACHE['nc']


def _prep_core_inputs(x, w_attn, b_attn, w_proj):
    """Build the 8 per-core input maps."""
    xts = [np.ascontiguousarray(x[b].T).astype(np.float32) for b in range(B)]
    in_maps = []
    tri = np.triu(np.ones((128, 128), dtype=np.float32))
    for c in range(NCORES):
        b = c // 2
        half = c % 2
        heads = [HPC * half + j for j in range(HPC)]
        wqk = np.empty((C, 768), dtype=np.float32)
        bqk = np.empty((768,), dtype=np.float32)
        wv = np.zeros((C, FV), dtype=np.float32)
        wvb = np.zeros((1, FV), dtype=np.float32)
        wp = np.empty((HPC * DH, 768), dtype=np.float32)
        for j, h in enumerate(heads):
            wqk[:, 64 * j:64 * j + 64] = w_attn[:, 64 * h:64 * h + 64] * 0.125
            bqk[64 * j:64 * j + 64] = b_attn[64 * h:64 * h + 64] * 0.125
            wqk[:, 384 + 64 * j:384 + 64 * j + 64] = w_attn[:, C + 64 * h:C + 64 * h + 64]
            bqk[384 + 64 * j:384 + 64 * j + 64] = b_attn[C + 64 * h:C + 64 * h + 64]
            wv[:, 65 * j:65 * j + 64] = w_attn[:, 2 * C + 64 * h:2 * C + 64 * h + 64]
            wvb[0, 65 * j:65 * j + 64] = b_attn[2 * C + 64 * h:2 * C + 64 * h + 64]
            wvb[0, 65 * j + 64] = 1.0
            wp[64 * j:64 * j + 64, :] = w_proj[64 * h:64 * h + 64, :]
        in_maps.append({
            "xt": xts[b], "wqk": wqk, "bqk": bqk, "wv": wv, "wvb": wvb,
            "wp": wp, "tri": tri,
        })
    return in_maps


def kernel(x, w_attn, b_attn, w_proj, b_proj):
    import sys
    if '/opt/trn_rl_repo' not in sys.path:
        sys.path.insert(0, '/opt/trn_rl_repo')
    from concourse.bass_utils import run_bass_kernel_spmd

    x = np.asarray(x, dtype=np.float32)
    w_attn = np.asarray(w_attn, dtype=np.float32)
    b_attn = np.asarray(b_attn, dtype=np.float32)
    w_proj = np.asarray(w_proj, dtype=np.float32)
    b_proj = np.asarray(b_proj, dtype=np.float32)

    nc = _get_nc()
    in_maps = _prep_core_inputs(x, w_attn, b_attn, w_proj)
    res = run_bass_kernel_spmd(nc, in_maps, list(range(NCORES))).results

    out = np.empty((B, T, C), dtype=np.float32)
    for b in range(B):
        out[b] = res[2 * b]["out"] + res[2 * b + 1]["out"] + b_proj
    return out


# revision 4
# speedup vs baseline: 1.2180x; 1.2180x over previous
"""Causal multi-head attention block (GPT-style) on 8 Trainium2 NeuronCores.

Sharding: 8 cores = 4 batches x 2 head-groups. Core c handles batch c//2 and
heads [6*(c%2), 6*(c%2)+6) of 12. Each core computes qkv projection, causal
attention and its partial output projection; the host sums the two partials
per batch and adds b_proj.

All matmuls run in float32r (TF32-like: ~1.7e-4 max rel err, full PE rate at
N>=256). q/k are produced feature-major ([d, T]) so QK^T needs no transposes;
v is position-major with a fused ones-column so the PV matmul also emits the
softmax denominator. Softmax skips max-subtraction (logits are ~N(0,1); exp
overflow impossible in fp32). Causality: per key-chunk column ranges plus one
triangular-mask multiply on the diagonal 128x128 sub-block.

Scheduling: engines execute their instruction streams in order, so emission
order is the schedule. The per-head key-chunk loop is software-pipelined with
a 2-chunk skew (QK runs 2 chunks ahead of PV) so the PE never stalls on the
ScalarE exp, and the qkv/v/output-projection matmuls of neighboring query
tiles are interleaved between heads as PE filler while ScalarE streams exps.
"""

import numpy as np

B, T, C = 4, 2048, 768
H = 12
DH = 64
HPC = 6          # heads per core
NCORES = 8
QT = 512         # query tile
NQT = T // QT    # 4
NKC = T // 128   # 16 key chunks
FV = HPC * (DH + 1)  # 390: per-head 64 v cols + 1 ones col

_CACHE = {}


def _build():
    import sys
    if '/opt/trn_rl_repo' not in sys.path:
        sys.path.insert(0, '/opt/trn_rl_repo')
    import concourse.tile as tile
    from concourse import bacc, mybir

    F32 = mybir.dt.float32
    F32R = mybir.dt.float32r
    EXP = mybir.ActivationFunctionType.Exp

    nc = bacc.Bacc("TRN2", target_bir_lowering=False, debug=False,
                   num_devices=NCORES)

    xt_ext = nc.dram_tensor("xt", [C, T], F32R, kind="ExternalInput").ap()
    wqk_ext = nc.dram_tensor("wqk", [C, 768], F32R, kind="ExternalInput").ap()
    bqk_ext = nc.dram_tensor("bqk", [768], F32, kind="ExternalInput").ap()
    wv_ext = nc.dram_tensor("wv", [C, FV], F32R, kind="ExternalInput").ap()
    wvb_ext = nc.dram_tensor("wvb", [1, FV], F32R, kind="ExternalInput").ap()
    wp_ext = nc.dram_tensor("wp", [HPC * DH, 768], F32R, kind="ExternalInput").ap()
    tri_ext = nc.dram_tensor("tri", [128, 128], F32, kind="ExternalInput").ap()
    out_ext = nc.dram_tensor("out", [T, 768], F32, kind="ExternalOutput").ap()

    xt_r = xt_ext.rearrange("(c p) n -> p c n", p=128)      # [128, 6, 2048]
    wqk_r = wqk_ext.rearrange("(c p) f -> p c f", p=128)    # [128, 6, 768]
    bqk_r = bqk_ext.rearrange("(c p) -> p c", p=128)        # [128, 6]
    wv_r = wv_ext.rearrange("(c p) f -> p c f", p=128)      # [128, 6, 390]
    wp_r = wp_ext.rearrange("(c p) f -> p c f", p=128)      # [128, 3, 768]

    with tile.TileContext(nc) as tc:
        import contextlib
        stack = contextlib.ExitStack()
        with stack:
            const = stack.enter_context(tc.tile_pool(name="const", bufs=1))
            big = stack.enter_context(tc.tile_pool(name="big", bufs=1))
            xtp = stack.enter_context(tc.tile_pool(name="xtp", bufs=14))
            expp = stack.enter_context(tc.tile_pool(name="expp", bufs=3))
            work = stack.enter_context(tc.tile_pool(name="work", bufs=2))
            psA = stack.enter_context(tc.tile_pool(name="psA", bufs=3, space="PSUM"))
            psY = stack.enter_context(tc.tile_pool(name="psY", bufs=2, space="PSUM"))
            psM = stack.enter_context(tc.tile_pool(name="psM", bufs=3, space="PSUM"))

            # ---- constants / weights (chunked DMAs so compute starts early) ----
            wqk_t = const.tile([128, 6, 768], F32R, tag="wqk")
            for cc in range(6):
                nc.sync.dma_start(out=wqk_t[:, cc, :], in_=wqk_r[:, cc, :])
            bqk_t = const.tile([128, 6], F32, tag="bqk")
            nc.sync.dma_start(out=bqk_t, in_=bqk_r)
            wv_t = const.tile([128, 6, FV], F32R, tag="wv")
            for cc in range(6):
                nc.sync.dma_start(out=wv_t[:, cc, :], in_=wv_r[:, cc, :])
            wvb_t = const.tile([1, FV], F32R, tag="wvb")
            nc.sync.dma_start(out=wvb_t, in_=wvb_ext)
            wp_t = const.tile([128, 3, 768], F32R, tag="wp")
            nc.sync.dma_start(out=wp_t, in_=wp_r)
            tri_t = const.tile([128, 128], F32, tag="tri")
            nc.sync.dma_start(out=tri_t, in_=tri_ext)
            ones_f32 = const.tile([1, T], F32, tag="ones_f32")
            nc.vector.memset(ones_f32, 1.0)
            ones_row = const.tile([1, T], F32R, tag="ones_row")
            nc.vector.tensor_copy(out=ones_row, in_=ones_f32)
            ones64 = const.tile([1, 64], F32R, tag="ones64")
            nc.vector.tensor_copy(out=ones64, in_=ones_f32[:, 0:64])

            # ---- persistent activations ----
            # qkT[fc]: fc 0..2 = q heads (head h -> tile h//2, partitions
            # 64*(h%2)..), fc 3..5 = k heads. All feature-major [128, T].
            qkT = [big.tile([128, T], F32R, tag=f"qkT{fc}", name=f"qkT{fc}") for fc in range(6)]
            v_t = [big.tile([128, FV], F32R, tag=f"v{kc}", name=f"v{kc}") for kc in range(NKC)]
            yT = [big.tile([128, T], F32R, tag=f"yT{kc}", name=f"yT{kc}") for kc in range(3)]

            xt_tiles = {}  # qt -> [6 tiles]

            def dma_xt(qt):
                tiles = []
                for cc in range(6):
                    xt_tl = xtp.tile([128, QT], F32R, tag="xt", name=f"xt_{qt}_{cc}")
                    nc.sync.dma_start(out=xt_tl,
                                      in_=xt_r[:, cc, qt * QT:(qt + 1) * QT])
                    tiles.append(xt_tl)
                xt_tiles[qt] = tiles

            def emit_B(qt, fc):
                """qk^T projection: one feature chunk of one query tile."""
                xts = xt_tiles[qt]
                ps = psM.tile([128, QT], F32, tag="mm", name=f"psB_{qt}_{fc}")
                for cc in range(6):
                    nc.tensor.matmul(out=ps,
                                     lhsT=wqk_t[:, cc, fc * 128:(fc + 1) * 128],
                                     rhs=xts[cc],
                                     start=(cc == 0), stop=(cc == 5))
                nc.vector.tensor_scalar(out=qkT[fc][:, qt * QT:(qt + 1) * QT],
                                        in0=ps, scalar1=bqk_t[:, fc:fc + 1],
                                        scalar2=None, op0=mybir.AluOpType.add)

            def emit_C(qt, tv):
                """v projection (position-major + ones col): one 128-row chunk."""
                xts = xt_tiles[qt]
                ps = psM.tile([128, FV], F32, tag="mm", name=f"psC_{tv}")
                for cc in range(6):
                    nc.tensor.matmul(out=ps,
                                     lhsT=xts[cc][:, (tv - 4 * qt) * 128:(tv - 4 * qt + 1) * 128],
                                     rhs=wv_t[:, cc, :],
                                     start=(cc == 0), stop=False)
                nc.tensor.matmul(out=ps,
                                 lhsT=ones_row[:, tv * 128:(tv + 1) * 128],
                                 rhs=wvb_t,
                                 start=False, stop=True)
                nc.vector.tensor_copy(out=v_t[tv], in_=ps)

            def emit_E(tt):
                """output projection for one 128-row chunk of t."""
                osb = work.tile([128, 768], F32, tag="osb", name=f"osb_{tt}")
                for no in range(2):
                    ne = 512 if no == 0 else 256
                    ps = psM.tile([128, QT], F32, tag="mm", name=f"psE_{tt}_{no}")
                    for kc in range(3):
                        nc.tensor.matmul(
                            out=ps[:, 0:ne],
                            lhsT=yT[kc][:, tt * 128:(tt + 1) * 128],
                            rhs=wp_t[:, kc, no * 512:no * 512 + ne],
                            start=(kc == 0), stop=(kc == 2))
                    nc.vector.tensor_copy(out=osb[:, no * 512:no * 512 + ne],
                                          in_=ps[:, 0:ne])
                nc.sync.dma_start(out=out_ext[tt * 128:(tt + 1) * 128, :],
                                  in_=osb)

            def emit_head(qt, h):
                """attention for one head: QK runs 2 key-chunks ahead of PV."""
                po = 64 * (h % 2)
                q_ap = qkT[h // 2][po:po + 64, qt * QT:(qt + 1) * QT]
                k_tl = qkT[3 + h // 2]
                psum_y = psY.tile([128, QT], F32, tag="y", name=f"psY_{qt}_{h}")
                nkc = 4 * qt + 4
                att = {}
                exps = {}

                def qk(k):
                    m = k - 4 * qt
                    lo = 128 * m if m >= 0 else 0
                    ps_att = psA.tile([128, QT], F32, tag="att", name=f"psA_{qt}_{h}_{k}")
                    nc.tensor.matmul(out=ps_att[:, lo:QT],
                                     lhsT=k_tl[po:po + 64, k * 128:(k + 1) * 128],
                                     rhs=q_ap[:, lo:QT],
                                     start=True, stop=True)
                    att[k] = (ps_att, lo)

                def ex(k):
                    ps_att, lo = att.pop(k)
                    e_t = expp.tile([128, QT], F32R, tag="expT", name=f"e_{qt}_{h}_{k}")
                    nc.scalar.activation(out=e_t[:, lo:QT], in_=ps_att[:, lo:QT],
                                         func=EXP)
                    if k - 4 * qt >= 0:
                        nc.vector.tensor_mul(e_t[:, lo:lo + 128],
                                             e_t[:, lo:lo + 128], tri_t)
                    exps[k] = (e_t, lo)

                def pv(k):
                    e_t, lo = exps.pop(k)
                    nc.tensor.matmul(out=psum_y[0:65, lo:QT],
                                     lhsT=v_t[k][:, 65 * h:65 * h + 65],
                                     rhs=e_t[:, lo:QT],
                                     start=(k == 0), stop=(k == nkc - 1),
                                     skip_group_check=True)

                qk(0)
                if nkc > 1:
                    qk(1)
                for k in range(nkc):
                    ex(k)
                    if k + 2 < nkc:
                        qk(k + 2)
                    pv(k)

                # normalize: broadcast den via ones outer product, recip, mul
                den_sb = work.tile([1, QT], F32R, tag="den", name=f"den_{qt}_{h}")
                nc.vector.tensor_copy(out=den_sb, in_=psum_y[64:65, :])
                ps_bc = psA.tile([64, QT], F32, tag="att", name=f"psBC_{qt}_{h}")
                nc.tensor.matmul(out=ps_bc, lhsT=ones64, rhs=den_sb,
                                 start=True, stop=True)
                recb = work.tile([64, QT], F32, tag="recb", name=f"recb_{qt}_{h}")
                nc.vector.reciprocal_approx_fast(out=recb, in_=ps_bc)
                nc.vector.tensor_mul(
                    yT[h // 2][po:po + 64, qt * QT:(qt + 1) * QT],
                    psum_y[0:64, :], recb)

            # ---- prologue: first query tile's projections ----
            dma_xt(0)
            for fc in range(6):
                emit_B(0, fc)
            for tv in range(4):
                emit_C(0, tv)

            # ---- main loop: attention with interleaved filler ----
            for qt in range(NQT):
                pending = []
                if qt < NQT - 1:
                    dma_xt(qt + 1)
                    pending += [lambda fc=fc: emit_B(qt + 1, fc) for fc in range(6)]
                    pending += [lambda tv=tv: emit_C(qt + 1, tv)
                                for tv in range(4 * qt + 4, 4 * qt + 8)]
                if qt > 0:
                    pending += [lambda tt=tt: emit_E(tt)
                                for tt in range(4 * qt - 4, 4 * qt)]
                for h in range(HPC):
                    emit_head(qt, h)
                    nshare = (len(pending) + HPC - 1 - h) // (HPC - h)
                    for _ in range(nshare):
                        if pending:
                            pending.pop(0)()
                for fn in pending:
                    fn()

            # ---- epilogue: last query tile's output projection ----
            for tt in range(T // 128 - 4, T // 128):
                emit_E(tt)

    nc.compile()
    return nc


def _get_nc():
    if 'nc' not in _CACHE:
        _CACHE['nc'] = _build()
    return _CACHE['nc']


def _prep_core_inputs(x, w_attn, b_attn, w_proj):
    """Build the 8 per-core input maps."""
    xts = [np.ascontiguousarray(x[b].T).astype(np.float32) for b in range(B)]
    in_maps = []
    tri = np.triu(np.ones((128, 128), dtype=np.float32))
    for c in range(NCORES):
        b = c // 2
        half = c % 2
        heads = [HPC * half + j for j in range(HPC)]
        wqk = np.empty((C, 768), dtype=np.float32)
        bqk = np.empty((768,), dtype=np.float32)
        wv = np.zeros((C, FV), dtype=np.float32)
        wvb = np.zeros((1, FV), dtype=np.float32)
        wp = np.empty((HPC * DH, 768), dtype=np.float32)
        for j, h in enumerate(heads):
            wqk[:, 64 * j:64 * j + 64] = w_attn[:, 64 * h:64 * h + 64] * 0.125
            bqk[64 * j:64 * j + 64] = b_attn[64 * h:64 * h + 64] * 0.125
            wqk[:, 384 + 64 * j:384 + 64 * j + 64] = w_attn[:, C + 64 * h:C + 64 * h + 64]
            bqk[384 + 64 * j:384 + 64 * j + 64] = b_attn[C + 64 * h:C + 64 * h + 64]
            wv[:, 65 * j:65 * j + 64] = w_attn[:, 2 * C + 64 * h:2 * C + 64 * h + 64]
            wvb[0, 65 * j:65 * j + 64] = b_attn[2 * C + 64 * h:2 * C + 64 * h + 64]
            wvb[0, 65 * j + 64] = 1.0
            wp[64 * j:64 * j + 64, :] = w_proj[64 * h:64 * h + 64, :]
        in_maps.append({
            "xt": xts[b], "wqk": wqk, "bqk": bqk, "wv": wv, "wvb": wvb,
            "wp": wp, "tri": tri,
        })
    return in_maps


def kernel(x, w_attn, b_attn, w_proj, b_proj):
    import sys
    if '/opt/trn_rl_repo' not in sys.path:
        sys.path.insert(0, '/opt/trn_rl_repo')
    from concourse.bass_utils import run_bass_kernel_spmd

    x = np.asarray(x, dtype=np.float32)
    w_attn = np.asarray(w_attn, dtype=np.float32)
    b_attn = np.asarray(b_attn, dtype=np.float32)
    w_proj = np.asarray(w_proj, dtype=np.float32)
    b_proj = np.asarray(b_proj, dtype=np.float32)

    nc = _get_nc()
    in_maps = _prep_core_inputs(x, w_attn, b_attn, w_proj)
    res = run_bass_kernel_spmd(nc, in_maps, list(range(NCORES))).results

    out = np.empty((B, T, C), dtype=np.float32)
    for b in range(B):
        out[b] = res[2 * b]["out"] + res[2 * b + 1]["out"] + b_proj
    return out


# revision 6
# speedup vs baseline: 1.4663x; 1.2039x over previous
"""Causal multi-head attention block (GPT-style) on 8 Trainium2 NeuronCores.

Sharding: 8 cores = 4 batches x 2 head-groups. Core c handles batch c//2 and
heads [6*(c%2), 6*(c%2)+6) of 12. Each core computes qkv projection, causal
attention and its partial output projection; the host sums the two partials
per batch and adds b_proj.

All matmuls run in float32r (TF32-like: ~1.7e-4 max rel err, full PE rate at
N>=256). q/k are produced feature-major ([d, T]) so QK^T needs no transposes;
v is position-major with a fused ones-column so the PV matmul also emits the
softmax denominator. Softmax skips max-subtraction (logits are ~N(0,1); exp
overflow impossible in fp32). Causality: per key-chunk column ranges plus one
triangular-mask multiply on the diagonal 128x128 sub-block.

Scheduling: engines execute their instruction streams in order, so emission
order is the schedule. The per-head key-chunk loop is software-pipelined with
a 2-chunk skew (QK runs 2 chunks ahead of PV) so the PE never stalls on the
ScalarE exp, and the qkv/v/output-projection matmuls of neighboring query
tiles are interleaved between heads as PE filler while ScalarE streams exps.
"""

import numpy as np

B, T, C = 4, 2048, 768
H = 12
DH = 64
HPC = 6          # heads per core
NCORES = 8
QT = 512         # query tile
NQT = T // QT    # 4
NKC = T // 128   # 16 key chunks
FV = HPC * (DH + 1)  # 390: per-head 64 v cols + 1 ones col

_CACHE = {}


def _build():
    import sys
    if '/opt/trn_rl_repo' not in sys.path:
        sys.path.insert(0, '/opt/trn_rl_repo')
    import concourse.tile as tile
    from concourse import bacc, mybir

    F32 = mybir.dt.float32
    F32R = mybir.dt.float32r
    EXP = mybir.ActivationFunctionType.Exp

    nc = bacc.Bacc("TRN2", target_bir_lowering=False, debug=False,
                   num_devices=NCORES)

    xt_ext = nc.dram_tensor("xt", [C, T], F32R, kind="ExternalInput").ap()
    wqk_ext = nc.dram_tensor("wqk", [C, 768], F32R, kind="ExternalInput").ap()
    bqk_ext = nc.dram_tensor("bqk", [768], F32, kind="ExternalInput").ap()
    wv_ext = nc.dram_tensor("wv", [C, FV], F32R, kind="ExternalInput").ap()
    wvb_ext = nc.dram_tensor("wvb", [1, FV], F32, kind="ExternalInput").ap()
    wp_ext = nc.dram_tensor("wp", [HPC * DH, 768], F32R, kind="ExternalInput").ap()
    tri_ext = nc.dram_tensor("tri", [128, 256], F32, kind="ExternalInput").ap()
    out_ext = nc.dram_tensor("out", [T, 768], F32, kind="ExternalOutput").ap()

    xt_r = xt_ext.rearrange("(c p) n -> p c n", p=128)      # [128, 6, 2048]
    wqk_r = wqk_ext.rearrange("(c p) f -> p c f", p=128)    # [128, 6, 768]
    bqk_r = bqk_ext.rearrange("(c p) -> p c", p=128)        # [128, 6]
    wv_r = wv_ext.rearrange("(c p) f -> p c f", p=128)      # [128, 6, 390]
    wp_r = wp_ext.rearrange("(c p) f -> p c f", p=128)      # [128, 3, 768]

    with tile.TileContext(nc) as tc:
        import contextlib
        stack = contextlib.ExitStack()
        with stack:
            const = stack.enter_context(tc.tile_pool(name="const", bufs=1))
            big = stack.enter_context(tc.tile_pool(name="big", bufs=1))
            xtp = stack.enter_context(tc.tile_pool(name="xtp", bufs=14))
            expp = stack.enter_context(tc.tile_pool(name="expp", bufs=3))
            work = stack.enter_context(tc.tile_pool(name="work", bufs=2))
            psA = stack.enter_context(tc.tile_pool(name="psA", bufs=3, space="PSUM"))
            psY = stack.enter_context(tc.tile_pool(name="psY", bufs=2, space="PSUM"))
            psM = stack.enter_context(tc.tile_pool(name="psM", bufs=3, space="PSUM"))

            # ---- constants / weights (chunked DMAs, highest priority first:
            # wqk+bqk+xt(0) gate phase B; wv gates C; wp/tri needed later) ----
            wqk_t = const.tile([128, 6, 768], F32R, tag="wqk")
            for cc in range(6):
                nc.sync.dma_start(out=wqk_t[:, cc, :], in_=wqk_r[:, cc, :])
            bqk_t = const.tile([128, 6], F32, tag="bqk")
            nc.sync.dma_start(out=bqk_t, in_=bqk_r)

            # ---- persistent activations ----
            # qkT[fc]: fc 0..2 = q heads (head h -> tile h//2, partitions
            # 64*(h%2)..), fc 3..5 = k heads. All feature-major [128, T].
            qkT = [big.tile([128, T], F32R, tag=f"qkT{fc}", name=f"qkT{fc}") for fc in range(6)]
            v_t = [big.tile([128, FV], F32R, tag=f"v{kc}", name=f"v{kc}") for kc in range(NKC)]
            yT = [big.tile([128, T], F32R, tag=f"yT{kc}", name=f"yT{kc}") for kc in range(3)]

            xt_tiles = {}  # qt -> [6 tiles]

            def dma_xt(qt):
                tiles = []
                for cc in range(6):
                    xt_tl = xtp.tile([128, QT], F32R, tag="xt", name=f"xt_{qt}_{cc}")
                    nc.sync.dma_start(out=xt_tl,
                                      in_=xt_r[:, cc, qt * QT:(qt + 1) * QT])
                    tiles.append(xt_tl)
                xt_tiles[qt] = tiles

            def emit_B(qt, fc):
                """qk^T projection: one feature chunk of one query tile."""
                xts = xt_tiles[qt]
                ps = psM.tile([128, QT], F32, tag="mm", name=f"psB_{qt}_{fc}")
                for cc in range(6):
                    nc.tensor.matmul(out=ps,
                                     lhsT=wqk_t[:, cc, fc * 128:(fc + 1) * 128],
                                     rhs=xts[cc],
                                     start=(cc == 0), stop=(cc == 5))
                nc.vector.tensor_scalar(out=qkT[fc][:, qt * QT:(qt + 1) * QT],
                                        in0=ps, scalar1=bqk_t[:, fc:fc + 1],
                                        scalar2=None, op0=mybir.AluOpType.add)

            def emit_C(qt, tv):
                """v projection (position-major + ones col): one 128-row chunk."""
                xts = xt_tiles[qt]
                ps = psM.tile([128, FV], F32, tag="mm", name=f"psC_{tv}")
                for cc in range(6):
                    nc.tensor.matmul(out=ps,
                                     lhsT=xts[cc][:, (tv - 4 * qt) * 128:(tv - 4 * qt + 1) * 128],
                                     rhs=wv_t[:, cc, :],
                                     start=(cc == 0), stop=(cc == 5))
                nc.vector.tensor_tensor(out=v_t[tv], in0=ps, in1=wvb_b,
                                        op=mybir.AluOpType.add)

            def emit_E(tt):
                """output projection for one 128-row chunk of t."""
                osb = work.tile([128, 768], F32, tag="osb", name=f"osb_{tt}")
                for no in range(2):
                    ne = 512 if no == 0 else 256
                    ps = psM.tile([128, QT], F32, tag="mm", name=f"psE_{tt}_{no}")
                    for kc in range(3):
                        nc.tensor.matmul(
                            out=ps[:, 0:ne],
                            lhsT=yT[kc][:, tt * 128:(tt + 1) * 128],
                            rhs=wp_t[:, kc, no * 512:no * 512 + ne],
                            start=(kc == 0), stop=(kc == 2))
                    nc.vector.tensor_copy(out=osb[:, no * 512:no * 512 + ne],
                                          in_=ps[:, 0:ne])
                nc.sync.dma_start(out=out_ext[tt * 128:(tt + 1) * 128, :],
                                  in_=osb)

            def emit_head(qt, h):
                """attention for one head: QK runs 2 key-chunks ahead of PV."""
                po = 64 * (h % 2)
                q_ap = qkT[h // 2][po:po + 64, qt * QT:(qt + 1) * QT]
                k_tl = qkT[3 + h // 2]
                psum_y = psY.tile([128, QT], F32, tag="y", name=f"psY_{qt}_{h}")
                nkc = 4 * qt + 4
                att = {}
                exps = {}

                def qk(k):
                    m = k - 4 * qt
                    lo = 128 * m if m >= 0 else 0
                    if m == 3:
                        lo = 256
                    ps_att = psA.tile([128, QT], F32, tag="att", name=f"psA_{qt}_{h}_{k}")
                    nc.tensor.matmul(out=ps_att[:, lo:QT],
                                     lhsT=k_tl[po:po + 64, k * 128:(k + 1) * 128],
                                     rhs=q_ap[:, lo:QT],
                                     start=True, stop=True)
                    att[k] = (ps_att, lo)

                def ex(k):
                    ps_att, lo = att.pop(k)
                    e_t = expp.tile([128, QT], F32R, tag="expT", name=f"e_{qt}_{h}_{k}")
                    nc.scalar.activation(out=e_t[:, lo:QT], in_=ps_att[:, lo:QT],
                                         func=EXP)
                    m = k - 4 * qt
                    if m == 3:
                        # cols [256:384) fully masked (zeros), [384:512) diag
                        nc.vector.tensor_mul(e_t[:, 256:512],
                                             e_t[:, 256:512], tri_t)
                    elif m >= 0:
                        nc.vector.tensor_mul(e_t[:, lo:lo + 128],
                                             e_t[:, lo:lo + 128],
                                             tri_t[:, 128:256])
                    exps[k] = (e_t, lo)

                def pv(k):
                    e_t, lo = exps.pop(k)
                    nc.tensor.matmul(out=psum_y[0:65, lo:QT],
                                     lhsT=v_t[k][:, 65 * h:65 * h + 65],
                                     rhs=e_t[:, lo:QT],
                                     start=(k == 0), stop=(k == nkc - 1),
                                     skip_group_check=True)

                qk(0)
                if nkc > 1:
                    qk(1)
                for k in range(nkc):
                    ex(k)
                    if k + 2 < nkc:
                        qk(k + 2)
                    pv(k)

                # normalize: gpsimd-broadcast den across 64 partitions,
                # reciprocal on DVE, scale the head output into yT
                den_sb = work.tile([1, QT], F32, tag="den", name=f"den_{qt}_{h}")
                nc.vector.tensor_copy(out=den_sb, in_=psum_y[64:65, :])
                den_b = work.tile([64, QT], F32, tag="den_b", name=f"den_b_{qt}_{h}")
                nc.gpsimd.partition_broadcast(out_ap=den_b, in_ap=den_sb)
                recb = work.tile([64, QT], F32, tag="recb", name=f"recb_{qt}_{h}")
                nc.vector.reciprocal_approx_fast(out=recb, in_=den_b)
                nc.vector.tensor_mul(
                    yT[h // 2][po:po + 64, qt * QT:(qt + 1) * QT],
                    psum_y[0:64, :], recb)

            # ---- prologue ----
            dma_xt(0)
            wv_t = const.tile([128, 6, FV], F32R, tag="wv")
            for cc in range(6):
                nc.sync.dma_start(out=wv_t[:, cc, :], in_=wv_r[:, cc, :])
            wvb_sb = const.tile([1, FV], F32, tag="wvb_sb")
            nc.sync.dma_start(out=wvb_sb, in_=wvb_ext)
            wvb_b = const.tile([128, FV], F32, tag="wvb_b")
            nc.gpsimd.partition_broadcast(out_ap=wvb_b, in_ap=wvb_sb)
            tri_t = const.tile([128, 256], F32, tag="tri")
            nc.sync.dma_start(out=tri_t, in_=tri_ext)
            wp_t = const.tile([128, 3, 768], F32R, tag="wp")
            nc.sync.dma_start(out=wp_t, in_=wp_r)
            emit_B(0, 0)
            emit_B(0, 3)
            for tv in range(4):
                emit_C(0, tv)
            for fc in (1, 4, 2, 5):
                emit_B(0, fc)

            # ---- main loop: attention with interleaved filler ----
            for qt in range(NQT):
                pending = []
                if qt < NQT - 1:
                    dma_xt(qt + 1)
                    pending += [lambda fc=fc: emit_B(qt + 1, fc) for fc in range(6)]
                    pending += [lambda tv=tv: emit_C(qt + 1, tv)
                                for tv in range(4 * qt + 4, 4 * qt + 8)]
                if qt > 0:
                    pending += [lambda tt=tt: emit_E(tt)
                                for tt in range(4 * qt - 4, 4 * qt)]
                for h in range(HPC):
                    emit_head(qt, h)
                    nshare = (len(pending) + HPC - 1 - h) // (HPC - h)
                    for _ in range(nshare):
                        if pending:
                            pending.pop(0)()
                for fn in pending:
                    fn()

            # ---- epilogue: last query tile's output projection ----
            for tt in range(T // 128 - 4, T // 128):
                emit_E(tt)

    nc.compile()
    return nc


def _get_nc():
    if 'nc' not in _CACHE:
        _CACHE['nc'] = _build()
    return _CACHE['nc']


def _prep_core_inputs(x, w_attn, b_attn, w_proj):
    """Build the 8 per-core input maps."""
    xts = [np.ascontiguousarray(x[b].T).astype(np.float32) for b in range(B)]
    in_maps = []
    tri = np.concatenate([np.zeros((128, 128), dtype=np.float32),
                          np.triu(np.ones((128, 128), dtype=np.float32))],
                         axis=1)
    for c in range(NCORES):
        b = c // 2
        half = c % 2
        heads = [HPC * half + j for j in range(HPC)]
        wqk = np.empty((C, 768), dtype=np.float32)
        bqk = np.empty((768,), dtype=np.float32)
        wv = np.zeros((C, FV), dtype=np.float32)
        wvb = np.zeros((1, FV), dtype=np.float32)
        wp = np.empty((HPC * DH, 768), dtype=np.float32)
        for j, h in enumerate(heads):
            wqk[:, 64 * j:64 * j + 64] = w_attn[:, 64 * h:64 * h + 64] * 0.125
            bqk[64 * j:64 * j + 64] = b_attn[64 * h:64 * h + 64] * 0.125
            wqk[:, 384 + 64 * j:384 + 64 * j + 64] = w_attn[:, C + 64 * h:C + 64 * h + 64]
            bqk[384 + 64 * j:384 + 64 * j + 64] = b_attn[C + 64 * h:C + 64 * h + 64]
            wv[:, 65 * j:65 * j + 64] = w_attn[:, 2 * C + 64 * h:2 * C + 64 * h + 64]
            wvb[0, 65 * j:65 * j + 64] = b_attn[2 * C + 64 * h:2 * C + 64 * h + 64]
            wvb[0, 65 * j + 64] = 1.0
            wp[64 * j:64 * j + 64, :] = w_proj[64 * h:64 * h + 64, :]
        in_maps.append({
            "xt": xts[b], "wqk": wqk, "bqk": bqk, "wv": wv, "wvb": wvb,
            "wp": wp, "tri": tri,
        })
    return in_maps


def kernel(x, w_attn, b_attn, w_proj, b_proj):
    import sys
    if '/opt/trn_rl_repo' not in sys.path:
        sys.path.insert(0, '/opt/trn_rl_repo')
    from concourse.bass_utils import run_bass_kernel_spmd

    x = np.asarray(x, dtype=np.float32)
    w_attn = np.asarray(w_attn, dtype=np.float32)
    b_attn = np.asarray(b_attn, dtype=np.float32)
    w_proj = np.asarray(w_proj, dtype=np.float32)
    b_proj = np.asarray(b_proj, dtype=np.float32)

    nc = _get_nc()
    in_maps = _prep_core_inputs(x, w_attn, b_attn, w_proj)
    res = run_bass_kernel_spmd(nc, in_maps, list(range(NCORES))).results

    out = np.empty((B, T, C), dtype=np.float32)
    for b in range(B):
        out[b] = res[2 * b]["out"] + res[2 * b + 1]["out"] + b_proj
    return out


# revision 7
# speedup vs baseline: 1.4794x; 1.0089x over previous
"""Causal multi-head attention block (GPT-style) on 8 Trainium2 NeuronCores.

Sharding: 8 cores = 4 batches x 2 head-groups. Core c handles batch c//2 and
heads [6*(c%2), 6*(c%2)+6) of 12. Each core computes qkv projection, causal
attention and its partial output projection; the host sums the two partials
per batch and adds b_proj.

All matmuls run in float32r (TF32-like: ~1.7e-4 max rel err, full PE rate at
N>=256). q/k are produced feature-major ([d, T]) so QK^T needs no transposes;
v is position-major with a fused ones-column so the PV matmul also emits the
softmax denominator. Softmax skips max-subtraction (logits are ~N(0,1); exp
overflow impossible in fp32). Causality: per key-chunk column ranges plus one
triangular-mask multiply on the diagonal 128x128 sub-block.

Scheduling: engines execute their instruction streams in order, so emission
order is the schedule. The per-head key-chunk loop is software-pipelined with
a 2-chunk skew (QK runs 2 chunks ahead of PV) so the PE never stalls on the
ScalarE exp, and the qkv/v/output-projection matmuls of neighboring query
tiles are interleaved between heads as PE filler while ScalarE streams exps.
"""

import numpy as np

B, T, C = 4, 2048, 768
H = 12
DH = 64
HPC = 6          # heads per core
NCORES = 8
QT = 512         # query tile
NQT = T // QT    # 4
NKC = T // 128   # 16 key chunks
FV = HPC * (DH + 1)  # 390: per-head 64 v cols + 1 ones col

_CACHE = {}


def _build():
    import sys
    if '/opt/trn_rl_repo' not in sys.path:
        sys.path.insert(0, '/opt/trn_rl_repo')
    import concourse.tile as tile
    from concourse import bacc, mybir

    F32 = mybir.dt.float32
    F32R = mybir.dt.float32r
    EXP = mybir.ActivationFunctionType.Exp

    nc = bacc.Bacc("TRN2", target_bir_lowering=False, debug=False,
                   num_devices=NCORES)

    xt_ext = nc.dram_tensor("xt", [C, T], F32R, kind="ExternalInput").ap()
    wqk_ext = nc.dram_tensor("wqk", [C, 768], F32R, kind="ExternalInput").ap()
    bqk_ext = nc.dram_tensor("bqk", [768], F32, kind="ExternalInput").ap()
    wv_ext = nc.dram_tensor("wv", [C, FV], F32R, kind="ExternalInput").ap()
    wvb_ext = nc.dram_tensor("wvb", [1, FV], F32, kind="ExternalInput").ap()
    wp_ext = nc.dram_tensor("wp", [HPC * DH, 768], F32R, kind="ExternalInput").ap()
    tri_ext = nc.dram_tensor("tri", [128, 256], F32, kind="ExternalInput").ap()
    out_ext = nc.dram_tensor("out", [T, 768], F32, kind="ExternalOutput").ap()

    xt_r = xt_ext.rearrange("(c p) n -> p c n", p=128)      # [128, 6, 2048]
    wqk_r = wqk_ext.rearrange("(c p) f -> p c f", p=128)    # [128, 6, 768]
    bqk_r = bqk_ext.rearrange("(c p) -> p c", p=128)        # [128, 6]
    wv_r = wv_ext.rearrange("(c p) f -> p c f", p=128)      # [128, 6, 390]
    wp_r = wp_ext.rearrange("(c p) f -> p c f", p=128)      # [128, 3, 768]

    with tile.TileContext(nc) as tc:
        import contextlib
        stack = contextlib.ExitStack()
        with stack:
            const = stack.enter_context(tc.tile_pool(name="const", bufs=1))
            big = stack.enter_context(tc.tile_pool(name="big", bufs=1))
            xtp = stack.enter_context(tc.tile_pool(name="xtp", bufs=14))
            expp = stack.enter_context(tc.tile_pool(name="expp", bufs=3))
            work = stack.enter_context(tc.tile_pool(name="work", bufs=2))
            psA = stack.enter_context(tc.tile_pool(name="psA", bufs=3, space="PSUM"))
            psY = stack.enter_context(tc.tile_pool(name="psY", bufs=2, space="PSUM"))
            psM = stack.enter_context(tc.tile_pool(name="psM", bufs=3, space="PSUM"))

            # ---- constants / weights (chunked DMAs, highest priority first:
            # wqk+bqk+xt(0) gate phase B; wv gates C; wp/tri needed later) ----
            wqk_t = const.tile([128, 6, 768], F32R, tag="wqk")
            bqk_t = const.tile([128, 6], F32, tag="bqk")
            nc.sync.dma_start(out=bqk_t, in_=bqk_r)

            # ---- persistent activations ----
            # qkT[fc]: fc 0..2 = q heads (head h -> tile h//2, partitions
            # 64*(h%2)..), fc 3..5 = k heads. All feature-major [128, T].
            qkT = [big.tile([128, T], F32R, tag=f"qkT{fc}", name=f"qkT{fc}") for fc in range(6)]
            v_t = [big.tile([128, FV], F32R, tag=f"v{kc}", name=f"v{kc}") for kc in range(NKC)]
            yT = [big.tile([128, T], F32R, tag=f"yT{kc}", name=f"yT{kc}") for kc in range(3)]

            xt_tiles = {}  # qt -> [6 tiles]

            def dma_xt(qt):
                tiles = []
                for cc in range(6):
                    xt_tl = xtp.tile([128, QT], F32R, tag="xt", name=f"xt_{qt}_{cc}")
                    nc.sync.dma_start(out=xt_tl,
                                      in_=xt_r[:, cc, qt * QT:(qt + 1) * QT])
                    tiles.append(xt_tl)
                xt_tiles[qt] = tiles

            def emit_B(qt, fc):
                """qk^T projection: one feature chunk of one query tile."""
                xts = xt_tiles[qt]
                ps = psM.tile([128, QT], F32, tag="mm", name=f"psB_{qt}_{fc}")
                for cc in range(6):
                    nc.tensor.matmul(out=ps,
                                     lhsT=wqk_t[:, cc, fc * 128:(fc + 1) * 128],
                                     rhs=xts[cc],
                                     start=(cc == 0), stop=(cc == 5))
                nc.vector.tensor_scalar(out=qkT[fc][:, qt * QT:(qt + 1) * QT],
                                        in0=ps, scalar1=bqk_t[:, fc:fc + 1],
                                        scalar2=None, op0=mybir.AluOpType.add)

            def emit_C(qt, tv):
                """v projection (position-major + ones col): one 128-row chunk."""
                xts = xt_tiles[qt]
                ps = psM.tile([128, FV], F32, tag="mm", name=f"psC_{tv}")
                for cc in range(6):
                    nc.tensor.matmul(out=ps,
                                     lhsT=xts[cc][:, (tv - 4 * qt) * 128:(tv - 4 * qt + 1) * 128],
                                     rhs=wv_t[:, cc, :],
                                     start=(cc == 0), stop=(cc == 5))
                nc.vector.tensor_tensor(out=v_t[tv], in0=ps, in1=wvb_b,
                                        op=mybir.AluOpType.add)

            def emit_E(tt):
                """output projection for one 128-row chunk of t."""
                osb = work.tile([128, 768], F32, tag="osb", name=f"osb_{tt}")
                for no in range(2):
                    ne = 512 if no == 0 else 256
                    ps = psM.tile([128, QT], F32, tag="mm", name=f"psE_{tt}_{no}")
                    for kc in range(3):
                        nc.tensor.matmul(
                            out=ps[:, 0:ne],
                            lhsT=yT[kc][:, tt * 128:(tt + 1) * 128],
                            rhs=wp_t[:, kc, no * 512:no * 512 + ne],
                            start=(kc == 0), stop=(kc == 2))
                    nc.vector.tensor_copy(out=osb[:, no * 512:no * 512 + ne],
                                          in_=ps[:, 0:ne])
                nc.sync.dma_start(out=out_ext[tt * 128:(tt + 1) * 128, :],
                                  in_=osb)

            def emit_head(qt, h):
                """attention for one head: QK runs 2 key-chunks ahead of PV."""
                po = 64 * (h % 2)
                q_ap = qkT[h // 2][po:po + 64, qt * QT:(qt + 1) * QT]
                k_tl = qkT[3 + h // 2]
                psum_y = psY.tile([128, QT], F32, tag="y", name=f"psY_{qt}_{h}")
                nkc = 4 * qt + 4
                att = {}
                exps = {}

                def qk(k):
                    m = k - 4 * qt
                    lo = 128 * m if m >= 0 else 0
                    if m == 3:
                        lo = 256
                    ps_att = psA.tile([128, QT], F32, tag="att", name=f"psA_{qt}_{h}_{k}")
                    nc.tensor.matmul(out=ps_att[:, lo:QT],
                                     lhsT=k_tl[po:po + 64, k * 128:(k + 1) * 128],
                                     rhs=q_ap[:, lo:QT],
                                     start=True, stop=True)
                    att[k] = (ps_att, lo)

                def ex(k):
                    ps_att, lo = att.pop(k)
                    e_t = expp.tile([128, QT], F32R, tag="expT", name=f"e_{qt}_{h}_{k}")
                    nc.scalar.activation(out=e_t[:, lo:QT], in_=ps_att[:, lo:QT],
                                         func=EXP)
                    m = k - 4 * qt
                    if m == 3:
                        # cols [256:384) fully masked (zeros), [384:512) diag
                        nc.vector.tensor_mul(e_t[:, 256:512],
                                             e_t[:, 256:512], tri_t)
                    elif m >= 0:
                        nc.vector.tensor_mul(e_t[:, lo:lo + 128],
                                             e_t[:, lo:lo + 128],
                                             tri_t[:, 128:256])
                    exps[k] = (e_t, lo)

                def pv(k):
                    e_t, lo = exps.pop(k)
                    nc.tensor.matmul(out=psum_y[0:65, lo:QT],
                                     lhsT=v_t[k][:, 65 * h:65 * h + 65],
                                     rhs=e_t[:, lo:QT],
                                     start=(k == 0), stop=(k == nkc - 1),
                                     skip_group_check=True)

                qk(0)
                if nkc > 1:
                    qk(1)
                for k in range(nkc):
                    ex(k)
                    if k + 2 < nkc:
                        qk(k + 2)
                    pv(k)

                # normalize: gpsimd-broadcast den across 64 partitions,
                # reciprocal on DVE, scale the head output into yT
                den_sb = work.tile([1, QT], F32, tag="den", name=f"den_{qt}_{h}")
                nc.vector.tensor_copy(out=den_sb, in_=psum_y[64:65, :])
                den_b = work.tile([64, QT], F32, tag="den_b", name=f"den_b_{qt}_{h}")
                nc.gpsimd.partition_broadcast(out_ap=den_b, in_ap=den_sb)
                recb = work.tile([64, QT], F32, tag="recb", name=f"recb_{qt}_{h}")
                nc.vector.reciprocal_approx_fast(out=recb, in_=den_b)
                nc.vector.tensor_mul(
                    yT[h // 2][po:po + 64, qt * QT:(qt + 1) * QT],
                    psum_y[0:64, :], recb)

            # ---- prologue ----
            for cc in range(6):
                nc.sync.dma_start(out=wqk_t[:, cc, :], in_=wqk_r[:, cc, :])
                xt_tl = xtp.tile([128, QT], F32R, tag="xt", name=f"xt_0_{cc}")
                nc.sync.dma_start(out=xt_tl, in_=xt_r[:, cc, 0:QT])
                xt_tiles.setdefault(0, []).append(xt_tl)
            wv_t = const.tile([128, 6, FV], F32R, tag="wv")
            for cc in range(6):
                nc.sync.dma_start(out=wv_t[:, cc, :], in_=wv_r[:, cc, :])
            wvb_sb = const.tile([1, FV], F32, tag="wvb_sb")
            nc.sync.dma_start(out=wvb_sb, in_=wvb_ext)
            wvb_b = const.tile([128, FV], F32, tag="wvb_b")
            nc.gpsimd.partition_broadcast(out_ap=wvb_b, in_ap=wvb_sb)
            tri_t = const.tile([128, 256], F32, tag="tri")
            nc.sync.dma_start(out=tri_t, in_=tri_ext)
            wp_t = const.tile([128, 3, 768], F32R, tag="wp")
            nc.sync.dma_start(out=wp_t, in_=wp_r)
            # cc-major first two B units: each matmul starts as soon as its
            # own wqk/xt chunk lands instead of waiting for all 12 DMAs
            ps_b0 = psM.tile([128, QT], F32, tag="mm", name="ps_b0")
            ps_b3 = psM.tile([128, QT], F32, tag="mm", name="ps_b3")
            for cc in range(6):
                nc.tensor.matmul(out=ps_b0, lhsT=wqk_t[:, cc, 0:128],
                                 rhs=xt_tiles[0][cc],
                                 start=(cc == 0), stop=(cc == 5))
                nc.tensor.matmul(out=ps_b3, lhsT=wqk_t[:, cc, 384:512],
                                 rhs=xt_tiles[0][cc],
                                 start=(cc == 0), stop=(cc == 5))
            nc.vector.tensor_scalar(out=qkT[0][:, 0:QT], in0=ps_b0,
                                    scalar1=bqk_t[:, 0:1], scalar2=None,
                                    op0=mybir.AluOpType.add)
            nc.vector.tensor_scalar(out=qkT[3][:, 0:QT], in0=ps_b3,
                                    scalar1=bqk_t[:, 3:4], scalar2=None,
                                    op0=mybir.AluOpType.add)
            for tv in range(4):
                emit_C(0, tv)
            for fc in (1, 4, 2, 5):
                emit_B(0, fc)

            # ---- main loop: attention with interleaved filler ----
            for qt in range(NQT):
                pending = []
                if qt < NQT - 1:
                    dma_xt(qt + 1)
                    pending += [lambda fc=fc: emit_B(qt + 1, fc) for fc in range(6)]
                    pending += [lambda tv=tv: emit_C(qt + 1, tv)
                                for tv in range(4 * qt + 4, 4 * qt + 8)]
                if qt > 0:
                    pending += [lambda tt=tt: emit_E(tt)
                                for tt in range(4 * qt - 4, 4 * qt)]
                for h in range(HPC):
                    emit_head(qt, h)
                    nshare = (len(pending) + HPC - 1 - h) // (HPC - h)
                    for _ in range(nshare):
                        if pending:
                            pending.pop(0)()
                for fn in pending:
                    fn()

            # ---- epilogue: last query tile's output projection ----
            for tt in range(T // 128 - 4, T // 128):
                emit_E(tt)

    nc.compile()
    return nc


def _get_nc():
    if 'nc' not in _CACHE:
        _CACHE['nc'] = _build()
    return _CACHE['nc']


def _prep_core_inputs(x, w_attn, b_attn, w_proj):
    """Build the 8 per-core input maps."""
    xts = [np.ascontiguousarray(x[b].T).astype(np.float32) for b in range(B)]
    in_maps = []
    tri = np.concatenate([np.zeros((128, 128), dtype=np.float32),
                          np.triu(np.ones((128, 128), dtype=np.float32))],
                         axis=1)
    for c in range(NCORES):
        b = c // 2
        half = c % 2
        heads = [HPC * half + j for j in range(HPC)]
        wqk = np.empty((C, 768), dtype=np.float32)
        bqk = np.empty((768,), dtype=np.float32)
        wv = np.zeros((C, FV), dtype=np.float32)
        wvb = np.zeros((1, FV), dtype=np.float32)
        wp = np.empty((HPC * DH, 768), dtype=np.float32)
        for j, h in enumerate(heads):
            wqk[:, 64 * j:64 * j + 64] = w_attn[:, 64 * h:64 * h + 64] * 0.125
            bqk[64 * j:64 * j + 64] = b_attn[64 * h:64 * h + 64] * 0.125
            wqk[:, 384 + 64 * j:384 + 64 * j + 64] = w_attn[:, C + 64 * h:C + 64 * h + 64]
            bqk[384 + 64 * j:384 + 64 * j + 64] = b_attn[C + 64 * h:C + 64 * h + 64]
            wv[:, 65 * j:65 * j + 64] = w_attn[:, 2 * C + 64 * h:2 * C + 64 * h + 64]
            wvb[0, 65 * j:65 * j + 64] = b_attn[2 * C + 64 * h:2 * C + 64 * h + 64]
            wvb[0, 65 * j + 64] = 1.0
            wp[64 * j:64 * j + 64, :] = w_proj[64 * h:64 * h + 64, :]
        in_maps.append({
            "xt": xts[b], "wqk": wqk, "bqk": bqk, "wv": wv, "wvb": wvb,
            "wp": wp, "tri": tri,
        })
    return in_maps


def kernel(x, w_attn, b_attn, w_proj, b_proj):
    import sys
    if '/opt/trn_rl_repo' not in sys.path:
        sys.path.insert(0, '/opt/trn_rl_repo')
    from concourse.bass_utils import run_bass_kernel_spmd

    x = np.asarray(x, dtype=np.float32)
    w_attn = np.asarray(w_attn, dtype=np.float32)
    b_attn = np.asarray(b_attn, dtype=np.float32)
    w_proj = np.asarray(w_proj, dtype=np.float32)
    b_proj = np.asarray(b_proj, dtype=np.float32)

    nc = _get_nc()
    in_maps = _prep_core_inputs(x, w_attn, b_attn, w_proj)
    res = run_bass_kernel_spmd(nc, in_maps, list(range(NCORES))).results

    out = np.empty((B, T, C), dtype=np.float32)
    for b in range(B):
        out[b] = res[2 * b]["out"] + res[2 * b + 1]["out"] + b_proj
    return out
